# revision 19
# baseline (speedup 1.0000x reference)
"""Trainium2 Bass kernel for the PaiNN-style EquivariantDecoder GNN.

Strategy (8 NeuronCores, node-partitioned graph parallel):
  * Nodes are permuted into 160 load-balanced blocks of 128 (20 per core) so
    each block receives an equal number of incoming directed edges.
  * Directed edges are assigned to the core/block owning their destination i.
  * Per layer: each core computes phi for its own nodes, AllGathers phi
    (bf16 rows) to HBM, gathers phi[j] / v[j] rows for its edges with
    dma_gather, forms the per-edge messages, and scatter-adds them into its
    own 128-node blocks with one-hot matmuls accumulated in PSUM.
  * States s, v are kept feature-major in SBUF in fp32; edge math in bf16.

Host-side work is strictly integer graph preprocessing and layout
(permutation, padding, transposes, dtype casts of inputs).
"""

import sys

if "/opt/trn_rl_repo" not in sys.path:
    sys.path.insert(0, "/opt/trn_rl_repo")

import math
import numpy as np

import concourse.bass as bass
import concourse.bacc as bacc
import concourse.tile as tile
import concourse.mybir as mybir
from concourse.masks import make_identity

F32 = mybir.dt.float32
BF16 = mybir.dt.bfloat16
I16 = mybir.dt.int16
I32 = mybir.dt.int32
AF = mybir.ActivationFunctionType
OP = mybir.AluOpType

BF16_NP = mybir.dt.np(BF16)


# ---------------------------------------------------------------------------
# configuration
# ---------------------------------------------------------------------------

def make_cfg(n_nodes, n_edges_u, bpc, t_u, ch, n_cores=8, f=128, nrbf=20,
             nconv=3, cutoff=20.0):
    t = bpc * t_u
    assert t % ch == 0
    return dict(
        N=n_nodes, EU=n_edges_u, NCORES=n_cores, F=f, F3=3 * f, NRBF=nrbf,
        NCONV=nconv, CUTOFF=cutoff, BPC=bpc, T_U=t_u, T=t, CH=ch,
        NCHUNK=t // ch, NLOC=bpc * 128, NSLOT=n_cores * bpc * 128,
        ECAP=t * 128,
    )


# full-size problem (hardcoded — kernel.py must be self-contained)
FULL_CFG = make_cfg(20000, 100000, bpc=20, t_u=10, ch=8)


# ---------------------------------------------------------------------------
# host-side graph preprocessing (pure integer / layout work)
# ---------------------------------------------------------------------------

def preprocess(nbr, cfg):
    import heapq

    n_nodes, n_cores, block = cfg["N"], cfg["NCORES"], 128
    nbr = np.asarray(nbr)
    src = nbr[:, 0].astype(np.int64)
    dst = nbr[:, 1].astype(np.int64)
    i_all = np.concatenate([src, dst])   # destination
    j_all = np.concatenate([dst, src])   # source
    deg = np.bincount(i_all, minlength=n_nodes)

    n_blocks = n_cores * cfg["BPC"]
    n_slots = n_blocks * block
    order = np.argsort(-deg, kind="stable")
    heap = [(0, b, 0) for b in range(n_blocks)]
    heapq.heapify(heap)
    perm = np.full(n_nodes, -1, dtype=np.int64)
    node_of = np.full(n_slots, -1, dtype=np.int64)
    fill = np.zeros(n_blocks, dtype=np.int64)
    esum = np.zeros(n_blocks, dtype=np.int64)
    for nid in order:
        while True:
            s, b, cnt = heapq.heappop(heap)
            if cnt == fill[b] and fill[b] < block:
                break
        perm[nid] = b * block + fill[b]
        node_of[b * block + fill[b]] = nid
        fill[b] += 1
        esum[b] += deg[nid]
        heapq.heappush(heap, (int(esum[b]), b, int(fill[b])))

    i_new = perm[i_all]
    j_new = perm[j_all]
    blk = i_new // block
    order_e = np.argsort(blk, kind="stable")
    cnts = np.bincount(blk, minlength=n_blocks)
    cap = cfg["T_U"] * block
    assert cnts.max() <= cap, (cnts.max(), cap)
    starts = np.zeros(n_blocks + 1, dtype=np.int64)
    np.cumsum(cnts, out=starts[1:])

    cores = []
    for c in range(n_cores):
        jv = np.zeros(cfg["ECAP"], dtype=np.int32)
        iv = np.zeros(cfg["ECAP"], dtype=np.int32)
        ev = np.full(cfg["ECAP"], -1, dtype=np.int64)
        for bb in range(cfg["BPC"]):
            b = c * cfg["BPC"] + bb
            sel = order_e[starts[b]:starts[b + 1]]
            n = sel.shape[0]
            base = bb * cap
            jv[base:base + n] = j_new[sel]
            iv[base:base + n] = i_new[sel] - b * block
            ev[base:base + n] = sel
        cores.append((jv, iv, ev))
    return dict(perm=perm, node_of=node_of, cores=cores,
                i_all=i_all, j_all=j_all)


def wrap_idx16(idx):
    idx = np.asarray(idx)
    n = idx.shape[0]
    assert n % 16 == 0
    out = np.zeros((128, n // 16), dtype=np.int16)
    out[:16, :] = idx.astype(np.int16).reshape(n // 16, 16).T
    return out


def build_inputs(cfg, pp, cg_xyz, cg_s, weights):
    """Returns in_maps: list of {name: np.ndarray} per core."""
    N, NLOC, T, ECAP = cfg["N"], cfg["NLOC"], cfg["T"], cfg["ECAP"]
    node_of = pp["node_of"]
    in_maps = []
    # per-layer weight tensors (identical across cores)
    wts = {}
    for l in range(cfg["NCONV"]):
        w1, b1 = weights["msg_W1"][l], weights["msg_b1"][l]
        w2, b2 = weights["msg_W2"][l], weights["msg_b2"][l]
        dw, db = weights["dist_W"][l], weights["dist_b"][l]
        uU, uV = weights["upd_U"][l], weights["upd_V"][l]
        sw1, sb1 = weights["upd_sW1"][l], weights["upd_sb1"][l]
        sw2, sb2 = weights["upd_sW2"][l], weights["upd_sb2"][l]
        dwbT = np.zeros((32, cfg["F3"]), dtype=np.float32)
        dwbT[:cfg["NRBF"]] = dw.T
        dwbT[cfg["NRBF"]] = db
        wts[f"w1T{l}"] = np.ascontiguousarray(w1.T).astype(BF16_NP)
        wts[f"b1_{l}"] = np.ascontiguousarray(b1[:, None]).astype(np.float32)
        wts[f"w2T{l}"] = np.ascontiguousarray(w2.T).astype(BF16_NP)
        wts[f"b2r{l}"] = np.ascontiguousarray(
            np.broadcast_to(b2[None, :], (128, b2.shape[0]))).astype(np.float32)
        wts[f"dwbT{l}"] = dwbT.astype(BF16_NP)
        wts[f"uUT{l}"] = np.ascontiguousarray(uU.T).astype(BF16_NP)
        wts[f"uVT{l}"] = np.ascontiguousarray(uV.T).astype(BF16_NP)
        wts[f"sw1Ta{l}"] = np.ascontiguousarray(sw1[:, :128].T).astype(BF16_NP)
        wts[f"sw1Tb{l}"] = np.ascontiguousarray(sw1[:, 128:].T).astype(BF16_NP)
        wts[f"sb1_{l}"] = np.ascontiguousarray(sb1[:, None]).astype(np.float32)
        wts[f"sw2T{l}"] = np.ascontiguousarray(sw2.T).astype(BF16_NP)
        wts[f"sb2c{l}"] = np.ascontiguousarray(
            sb2.reshape(3, 128).T).astype(np.float32)

    for c in range(cfg["NCORES"]):
        jv, iv, ev = pp["cores"][c]
        valid = ev >= 0
        # xyz per edge slot (host gather = pure data routing)
        xi = np.zeros((ECAP, 3), dtype=np.float32)
        xj = np.zeros((ECAP, 3), dtype=np.float32)
        xi[valid] = cg_xyz[pp["i_all"][ev[valid]]]
        xj[valid] = cg_xyz[pp["j_all"][ev[valid]]]
        xj[~valid, 0] = 1.0  # pad edges: d = 1, masked out via env
        # tile-major [128, T, 3] -> partition p holds edge (t*128+p)
        xi_t = xi.reshape(T, 128, 3).transpose(1, 0, 2).reshape(128, T * 3)
        xj_t = xj.reshape(T, 128, 3).transpose(1, 0, 2).reshape(128, T * 3)
        iloc_t = iv.reshape(T, 128).T.astype(np.float32)
        mask_t = valid.reshape(T, 128).T.astype(np.float32)
        # own-shard s, feature-major
        sl = node_of[c * NLOC:(c + 1) * NLOC]
        s0 = np.zeros((NLOC, cfg["F"]), dtype=np.float32)
        s0[sl >= 0] = cg_s[sl[sl >= 0]]
        m = dict(
            s0=np.ascontiguousarray(s0.T),
            xyzi=np.ascontiguousarray(xi_t),
            xyzj=np.ascontiguousarray(xj_t),
            iloc=np.ascontiguousarray(iloc_t),
            mask=np.ascontiguousarray(mask_t),
            jwrap=wrap_idx16(jv),
        )
        m.update(wts)
        in_maps.append(m)
    return in_maps


# ---------------------------------------------------------------------------
# device kernel
# ---------------------------------------------------------------------------

def build_graph(tc, outs, ins, cfg):
    """Build the full device program. outs/ins: dicts of DRAM APs."""
    nc = tc.nc
    N, F, F3 = cfg["N"], cfg["F"], cfg["F3"]
    NLOC, NSLOT, T, T_U, BPC = (cfg["NLOC"], cfg["NSLOT"], cfg["T"],
                                cfg["T_U"], cfg["BPC"])
    CH, NCHUNK, ECAP = cfg["CH"], cfg["NCHUNK"], cfg["ECAP"]
    NCONV, CUTOFF, NRBF = cfg["NCONV"], cfg["CUTOFF"], cfg["NRBF"]
    NCORES = cfg["NCORES"]
    RG = [list(range(NCORES))]
    NCH_NODE = (NLOC + 511) // 512  # node chunks for matmul phases

    # ---- internal DRAM ----
    phi_loc = nc.dram_tensor("phi_loc", [NLOC, F3], BF16, kind="Internal")
    phi_full = nc.dram_tensor("phi_full", [NSLOT, F3], BF16, kind="Internal",
                              addr_space="Shared")
    v_loc = nc.dram_tensor("v_loc", [NLOC, F3], BF16, kind="Internal")
    v_full = nc.dram_tensor("v_full", [NSLOT, F3], BF16, kind="Internal",
                            addr_space="Shared")
    renvT_dram = nc.dram_tensor("renvT_dram", [32, ECAP], BF16, kind="Internal")

    # ---- persistent SBUF ----
    from contextlib import ExitStack
    stack = ExitStack()
    pers = stack.enter_context(tc.tile_pool(name="pers", bufs=1))
    ident = pers.tile([128, 128], F32)
    make_identity(nc, ident[:])
    ident_b = pers.tile([128, 128], BF16)
    make_identity(nc, ident_b[:])
    iota_f = pers.tile([128, 128], F32)
    iota_i = pers.tile([128, 128], I32)
    nc.gpsimd.iota(iota_i[:], pattern=[[1, 128]], base=0, channel_multiplier=0)
    nc.vector.tensor_copy(iota_f[:], iota_i[:])

    halfpi = pers.tile([128, 1], F32)
    nc.gpsimd.memset(halfpi[:], math.pi / 2)

    s_st = pers.tile([128, NLOC], F32)
    v_st = pers.tile([128, 3 * NLOC], F32)
    sbf = pers.tile([128, NLOC], BF16)
    h1 = pers.tile([128, NLOC], BF16)
    vbf = pers.tile([128, 3 * NLOC], BF16)
    unit = pers.tile([128, T * 3], F32)
    iloc_s = pers.tile([128, T], F32)
    jw_s = pers.tile([128, ECAP // 16], I16)

    nc.sync.dma_start(s_st[:], ins["s0"][:])
    nc.gpsimd.memset(v_st[:], 0.0)
    nc.sync.dma_start(iloc_s[:], ins["iloc"][:])
    nc.sync.dma_start(jw_s[:], ins["jwrap"][:])

    wt = {}
    for l in range(NCONV):
        for nm, shp, dt in [
            (f"w1T{l}", [128, 128], BF16), (f"b1_{l}", [128, 1], F32),
            (f"w2T{l}", [128, F3], BF16), (f"b2r{l}", [128, F3], F32),
            (f"dwbT{l}", [32, F3], BF16), (f"uUT{l}", [128, 128], BF16),
            (f"uVT{l}", [128, 128], BF16), (f"sw1Ta{l}", [128, 128], BF16),
            (f"sw1Tb{l}", [128, 128], BF16), (f"sb1_{l}", [128, 1], F32),
            (f"sw2T{l}", [128, F3], BF16), (f"sb2c{l}", [128, 3], F32),
        ]:
            wt[nm] = pers.tile(shp, dt, tag=nm, name=nm)
            nc.sync.dma_start(wt[nm][:], ins[nm][:])

    # ================= static edge pass =================
    with tc.tile_pool(name="static", bufs=1) as stp, \
         tc.tile_pool(name="static_ps", bufs=2, space="PSUM") as stps, \
         tc.tile_pool(name="static_w", bufs=2) as stw:
        xyzi_s = stp.tile([128, T * 3], F32)
        xyzj_s = stp.tile([128, T * 3], F32)
        mask_s = stp.tile([128, T], F32)
        nc.sync.dma_start(xyzi_s[:], ins["xyzi"][:])
        nc.sync.dma_start(xyzj_s[:], ins["xyzj"][:])
        nc.sync.dma_start(mask_s[:], ins["mask"][:])

        r = stp.tile([128, T * 3], F32)
        nc.vector.tensor_sub(r[:], xyzj_s[:], xyzi_s[:])
        r2 = stp.tile([128, T * 3], F32)
        nc.vector.tensor_mul(r2[:], r[:], r[:])
        d2 = stp.tile([128, T], F32)
        nc.vector.tensor_reduce(
            d2[:], r2[:].rearrange("p (t k) -> p t k", k=3),
            axis=mybir.AxisListType.X, op=OP.add)
        d = stp.tile([128, T], F32)
        nc.scalar.activation(d[:], d2[:], AF.Sqrt)
        rinv = stp.tile([128, T], F32)
        nc.vector.reciprocal(rinv[:], d[:])
        nc.vector.tensor_mul(
            unit[:].rearrange("p (t k) -> p t k", k=3),
            r[:].rearrange("p (t k) -> p t k", k=3),
            rinv[:].to_broadcast([128, T, 3]))
        # envelope
        env = stp.tile([128, T], F32)
        # theta = pi*d/CUTOFF in [0, pi); cos via sin(pi/2 - theta) (in range)
        c1 = stp.tile([128, T], F32)
        nc.scalar.activation(c1[:], d[:], AF.Sin,
                             scale=-math.pi / CUTOFF, bias=halfpi[:])
        s1 = stp.tile([128, T], F32)
        nc.scalar.activation(s1[:], d[:], AF.Sin, scale=math.pi / CUTOFF)
        nc.vector.tensor_scalar(env[:], c1[:], 0.5, 0.5, OP.mult, OP.add)
        nc.vector.tensor_mul(env[:], env[:], mask_s[:])
        menv = stp.tile([128, T], F32)
        nc.vector.tensor_mul(menv[:], env[:], rinv[:])
        # rbf rows via Chebyshev: sin((k+1)t) = 2cos(t)sin(kt) - sin((k-1)t)
        c2t = stp.tile([128, T], F32)
        nc.vector.tensor_scalar(c2t[:], c1[:], 2.0, None, OP.mult)
        big = stp.tile([128, T * 32], F32)
        bigv = big[:].rearrange("p (t k) -> p t k", k=32)
        tmp = stp.tile([128, T], F32)
        nc.vector.tensor_copy(bigv[:, :, 0:1],
                              s1[:].rearrange("p (t k) -> p t k", k=1))
        nc.vector.tensor_mul(bigv[:, :, 1:2],
                             c2t[:].rearrange("p (t k) -> p t k", k=1),
                             bigv[:, :, 0:1])
        for kk in range(2, NRBF):
            nc.vector.tensor_mul(tmp[:].rearrange("p (t k) -> p t k", k=1),
                                 c2t[:].rearrange("p (t k) -> p t k", k=1),
                                 bigv[:, :, kk - 1:kk])
            nc.vector.tensor_sub(bigv[:, :, kk:kk + 1],
                                 tmp[:].rearrange("p (t k) -> p t k", k=1),
                                 bigv[:, :, kk - 2:kk - 1])
        nc.vector.tensor_mul(bigv[:, :, 0:NRBF], bigv[:, :, 0:NRBF],
                             menv[:].to_broadcast([128, T, NRBF]))
        nc.vector.tensor_copy(bigv[:, :, NRBF:NRBF + 1],
                              env[:].rearrange("p (t k) -> p t k", k=1))
        nc.vector.memset(bigv[:, :, NRBF + 1:], 0.0)
        # transpose each tile's [128, 32] -> [32, 128], store bf16 to DRAM
        for ck in range(NCHUNK):
            stg = stw.tile([32, CH * 128], BF16, tag="renv_stg")
            for k in range(CH):
                t = ck * CH + k
                ps = stps.tile([32, 128], F32, tag="renv_ps")
                nc.tensor.transpose(ps[:], big[:, t * 32:(t + 1) * 32],
                                    ident[:])
                nc.vector.tensor_copy(stg[:, k * 128:(k + 1) * 128], ps[:])
            nc.sync.dma_start(renvT_dram[:, ck * CH * 128:(ck + 1) * CH * 128],
                              stg[:])
            if "dbg_renv" in outs:
                nc.sync.dma_start(
                    outs["dbg_renv"][:, ck * CH * 128:(ck + 1) * CH * 128],
                    stg[:])

    # ================= pools for the main loop =================
    psum = stack.enter_context(tc.tile_pool(name="ps", bufs=3, space="PSUM"))
    acc_pool = stack.enter_context(
        tc.tile_pool(name="acc", bufs=2, space="PSUM"))
    wke = stack.enter_context(tc.tile_pool(name="wke", bufs=3))
    wk = stack.enter_context(tc.tile_pool(name="wk", bufs=2))
    gat = stack.enter_context(tc.tile_pool(name="gat", bufs=2))

    def cast_bf(dst, src):
        nc.vector.tensor_copy(dst, src)

    # ================= layers =================
    for l in range(NCONV):
        # ---- phi = silu(s @ W1^T + b1) @ W2^T + b2, own nodes ----
        cast_bf(sbf[:], s_st[:])
        for nch in range(NCH_NODE):
            ns = slice(nch * 512, min((nch + 1) * 512, NLOC))
            ph = psum.tile([128, 512], F32, tag="psB")
            w_ = ns.stop - ns.start
            nc.tensor.matmul(ph[:, :w_], wt[f"w1T{l}"][:],
                             sbf[:, ns], start=True, stop=True)
            zf = wk.tile([128, 512], BF16, tag="zf")
            nc.vector.tensor_scalar(zf[:, :w_], ph[:, :w_],
                                    wt[f"b1_{l}"][:, 0:1], None, OP.add)
            sg = wk.tile([128, 512], BF16, tag="sg")
            nc.scalar.activation(sg[:, :w_], zf[:, :w_], AF.Sigmoid)
            nc.vector.tensor_mul(h1[:, ns], zf[:, :w_], sg[:, :w_])
        for nt in range(BPC):
            pp_ = psum.tile([128, F3], F32, tag="psB", name="pp_")
            nc.tensor.matmul(pp_[:], h1[:, nt * 128:(nt + 1) * 128],
                             wt[f"w2T{l}"][:], start=True, stop=True)
            phs = wk.tile([128, F3], BF16, tag="phs")
            nc.vector.tensor_add(phs[:], pp_[:], wt[f"b2r{l}"][:])
            nc.sync.dma_start(phi_loc[nt * 128:(nt + 1) * 128, :], phs[:])
        nc.gpsimd.collective_compute(
            "AllGather", OP.bypass, replica_groups=RG,
            ins=[phi_loc.ap()], outs=[phi_full.ap()])
        if l == 0 and "dbg_phi" in outs:
            nc.sync.dma_start(outs["dbg_phi"][:], phi_full.ap())

        # ---- edge message pass ----
        for ck in range(NCHUNK):
            ce = slice(ck * CH * 128, (ck + 1) * CH * 128)
            pj = gat.tile([128, CH, F3], BF16, tag="pj")
            nc.gpsimd.dma_gather(
                out_ap=pj[:], in_ap=phi_full.ap(), idxs_ap=jw_s[:, ck * CH * 8:(ck + 1) * CH * 8],
                num_idxs=CH * 128, num_idxs_reg=CH * 128, elem_size=F3)
            if l > 0:
                vj = gat.tile([128, CH, F3], BF16, tag="vj")
                nc.gpsimd.dma_gather(
                    out_ap=vj[:], in_ap=v_full.ap(),
                    idxs_ap=jw_s[:, ck * CH * 8:(ck + 1) * CH * 8],
                    num_idxs=CH * 128, num_idxs_reg=CH * 128, elem_size=F3)
            rT = gat.tile([32, CH * 128], BF16, tag="rT")
            nc.sync.dma_start(rT[:], renvT_dram.ap()[:, ce])

            for k in range(CH):
                t = ck * CH + k
                b = t // T_U
                first = (t % T_U == 0)
                last = (t % T_U == T_U - 1)
                if first:
                    acc = acc_pool.tile([128, 512], F32, tag="acc")
                ws = psum.tile([128, F3], F32, tag="ws", bufs=2)
                nc.tensor.matmul(ws[:], rT[0:NRBF + 1, k * 128:(k + 1) * 128],
                                 wt[f"dwbT{l}"][0:NRBF + 1, :],
                                 start=True, stop=True)
                P = wke.tile([128, 128], BF16, tag="P")
                nc.vector.tensor_scalar(P[:], iota_f[:], iloc_s[:, t:t + 1],
                                        None, OP.is_equal)
                buf1 = wke.tile([128, 512], BF16, tag="buf1")
                nc.vector.tensor_mul(buf1[:, 0:128], pj[:, k, 128:256],
                                     ws[:, 128:256])
                inv0 = wke.tile([128, 128], BF16, tag="inv0")
                nc.vector.tensor_mul(inv0[:], pj[:, k, 0:128], ws[:, 0:128])
                inv2 = wke.tile([128, 128], BF16, tag="inv2")
                nc.vector.tensor_mul(inv2[:], pj[:, k, 256:384],
                                     ws[:, 256:384])
                uv3 = unit[:].rearrange("p (t k) -> p t k", k=3)
                for dd in range(3):
                    nc.vector.tensor_scalar(
                        buf1[:, 128 * (dd + 1):128 * (dd + 2)], inv2[:],
                        uv3[:, t, dd:dd + 1], None, OP.mult)
                nc.tensor.matmul(acc[:], P[:], buf1[:], start=first,
                                 stop=(last and l == 0))
                if l > 0:
                    buf2 = wke.tile([128, F3], BF16, tag="buf2")
                    for dd in range(3):
                        nc.gpsimd.tensor_mul(buf2[:, dd * 128:(dd + 1) * 128],
                                             inv0[:],
                                             vj[:, k, dd * 128:(dd + 1) * 128])
                    nc.tensor.matmul(acc[:, 128:512], P[:], buf2[:],
                                     start=False, stop=last)
                if last:
                    stg = wk.tile([128, 512], F32, tag="dstg")
                    nc.vector.tensor_copy(stg[:], acc[:])
                    ps2 = psum.tile([128, 512], F32, tag="ps2", bufs=1)
                    for q in range(4):
                        nc.tensor.transpose(ps2[:, q * 128:(q + 1) * 128],
                                            stg[:, q * 128:(q + 1) * 128],
                                            ident[:])
                    nb = slice(b * 128, (b + 1) * 128)
                    nc.vector.tensor_add(s_st[:, nb], s_st[:, nb],
                                         ps2[:, 0:128])
                    for dd in range(3):
                        vs = slice(dd * NLOC + b * 128, dd * NLOC + (b + 1) * 128)
                        nc.vector.tensor_add(v_st[:, vs], v_st[:, vs],
                                             ps2[:, (dd + 1) * 128:(dd + 2) * 128])

        if l == 0 and "dbg_smsg" in outs:
            nc.sync.dma_start(outs["dbg_smsg"][:], s_st[:])
            nc.sync.dma_start(outs["dbg_vmsg"][:], v_st[:])

        # ---- update block ----
        cast_bf(vbf[:], v_st[:])
        for nch in range(NCH_NODE):
            lo = nch * 512
            hi = min((nch + 1) * 512, NLOC)
            w = hi - lo
            ns = slice(lo, hi)
            uv_sb = []
            for dd in range(3):
                up = psum.tile([128, 512], F32, tag="psB", name="up")
                nc.tensor.matmul(up[:, :w], wt[f"uUT{l}"][:],
                                 vbf[:, dd * NLOC + lo:dd * NLOC + hi],
                                 start=True, stop=True)
                u_sb = wk.tile([128, 512], BF16, tag=f"uv{dd}")
                nc.vector.tensor_copy(u_sb[:, :w], up[:, :w])
                uv_sb.append(u_sb)
            vv_sb = []
            for dd in range(3):
                vp = psum.tile([128, 512], F32, tag="psB", name="vp")
                nc.tensor.matmul(vp[:, :w], wt[f"uVT{l}"][:],
                                 vbf[:, dd * NLOC + lo:dd * NLOC + hi],
                                 start=True, stop=True)
                v_sb = wk.tile([128, 512], BF16, tag=f"vv{dd}")
                nc.vector.tensor_copy(v_sb[:, :w], vp[:, :w])
                vv_sb.append(v_sb)
            vv2 = wk.tile([128, 512], BF16, tag="vv2")
            tq = wk.tile([128, 512], BF16, tag="tq")
            nc.vector.tensor_mul(vv2[:, :w], vv_sb[0][:, :w], vv_sb[0][:, :w])
            nc.vector.tensor_mul(tq[:, :w], vv_sb[1][:, :w], vv_sb[1][:, :w])
            nc.vector.tensor_add(vv2[:, :w], vv2[:, :w], tq[:, :w])
            nc.vector.tensor_mul(tq[:, :w], vv_sb[2][:, :w], vv_sb[2][:, :w])
            nc.vector.tensor_add(vv2[:, :w], vv2[:, :w], tq[:, :w])
            nrm = wk.tile([128, 512], BF16, tag="nrm")
            nc.scalar.activation(nrm[:, :w], vv2[:, :w], AF.Sqrt)
            # dot3 = sum_d uv_d * vv_d
            dot = wk.tile([128, 512], BF16, tag="dot")
            dtm = wk.tile([128, 512], BF16, tag="dtm")
            nc.vector.tensor_mul(dot[:, :w], uv_sb[0][:, :w], vv_sb[0][:, :w])
            nc.vector.tensor_mul(dtm[:, :w], uv_sb[1][:, :w], vv_sb[1][:, :w])
            nc.vector.tensor_add(dot[:, :w], dot[:, :w], dtm[:, :w])
            nc.vector.tensor_mul(dtm[:, :w], uv_sb[2][:, :w], vv_sb[2][:, :w])
            nc.vector.tensor_add(dot[:, :w], dot[:, :w], dtm[:, :w])
            # a-MLP
            smid = wk.tile([128, 512], BF16, tag="smid")
            nc.vector.tensor_copy(smid[:, :w], s_st[:, ns])
            hp = psum.tile([128, 512], F32, tag="psB", name="hp")
            nc.tensor.matmul(hp[:, :w], wt[f"sw1Ta{l}"][:], smid[:, :w],
                             start=True, stop=False)
            nc.tensor.matmul(hp[:, :w], wt[f"sw1Tb{l}"][:], nrm[:, :w],
                             start=False, stop=True)
            hsb = wk.tile([128, 512], BF16, tag="hsb")
            zf2 = wk.tile([128, 512], BF16, tag="zf")
            nc.vector.tensor_scalar(zf2[:, :w], hp[:, :w],
                                    wt[f"sb1_{l}"][:, 0:1], None, OP.add)
            sg2 = wk.tile([128, 512], BF16, tag="sg")
            nc.scalar.activation(sg2[:, :w], zf2[:, :w], AF.Sigmoid)
            nc.vector.tensor_mul(hsb[:, :w], zf2[:, :w], sg2[:, :w])
            a_sb = []
            for cc in range(3):
                ap_ = psum.tile([128, 512], F32, tag="psB", name="ap_")
                nc.tensor.matmul(ap_[:, :w],
                                 wt[f"sw2T{l}"][:, cc * 128:(cc + 1) * 128],
                                 hsb[:, :w], start=True, stop=True)
                g_sb = wk.tile([128, 512], BF16, tag=f"a{cc}")
                nc.vector.tensor_scalar(g_sb[:, :w], ap_[:, :w],
                                        wt[f"sb2c{l}"][:, cc:cc + 1], None,
                                        OP.add)
                a_sb.append(g_sb)
            # s += dot * a1 + a2
            t1 = wk.tile([128, 512], BF16, tag="t1")
            nc.vector.tensor_mul(t1[:, :w], dot[:, :w], a_sb[1][:, :w])
            nc.vector.tensor_add(s_st[:, ns], s_st[:, ns], t1[:, :w])
            nc.vector.tensor_add(s_st[:, ns], s_st[:, ns], a_sb[2][:, :w])
            # v += u_v * a0
            for dd in range(3):
                nc.vector.tensor_mul(t1[:, :w], uv_sb[dd][:, :w],
                                     a_sb[0][:, :w])
                vs = slice(dd * NLOC + lo, dd * NLOC + hi)
                nc.vector.tensor_add(v_st[:, vs], v_st[:, vs], t1[:, :w])

        # ---- publish v rows for next layer's gather ----
        if l < NCONV - 1:
            cast_bf(vbf[:], v_st[:])
            for nt in range(BPC):
                vrow = wk.tile([128, F3], BF16, tag="vrow")
                for dd in range(3):
                    pv = psum.tile([128, 128], BF16, tag="psB", name="pv")
                    nc.tensor.transpose(
                        pv[:],
                        vbf[:, dd * NLOC + nt * 128:dd * NLOC + (nt + 1) * 128],
                        ident_b[:])
                    nc.vector.tensor_copy(vrow[:, dd * 128:(dd + 1) * 128],
                                          pv[:])
                nc.sync.dma_start(v_loc[nt * 128:(nt + 1) * 128, :], vrow[:])
            nc.gpsimd.collective_compute(
                "AllGather", OP.bypass, replica_groups=RG,
                ins=[v_loc.ap()], outs=[v_full.ap()])

    # ---- outputs ----
    nc.sync.dma_start(outs["out_s"][:], s_st[:])
    nc.sync.dma_start(outs["out_v"][:], v_st[:])

    stack.close()


# ---------------------------------------------------------------------------
# entry point
# ---------------------------------------------------------------------------

def _input_specs(cfg):
    specs = dict(
        s0=([128, cfg["NLOC"]], F32),
        xyzi=([128, cfg["T"] * 3], F32),
        xyzj=([128, cfg["T"] * 3], F32),
        iloc=([128, cfg["T"]], F32),
        mask=([128, cfg["T"]], F32),
        jwrap=([128, cfg["ECAP"] // 16], I16),
    )
    for l in range(cfg["NCONV"]):
        specs.update({
            f"w1T{l}": ([128, 128], BF16), f"b1_{l}": ([128, 1], F32),
            f"w2T{l}": ([128, cfg["F3"]], BF16), f"b2r{l}": ([128, cfg["F3"]], F32),
            f"dwbT{l}": ([32, cfg["F3"]], BF16), f"uUT{l}": ([128, 128], BF16),
            f"uVT{l}": ([128, 128], BF16), f"sw1Ta{l}": ([128, 128], BF16),
            f"sw1Tb{l}": ([128, 128], BF16), f"sb1_{l}": ([128, 1], F32),
            f"sw2T{l}": ([128, cfg["F3"]], BF16), f"sb2c{l}": ([128, 3], F32),
        })
    return specs


def build_bass(cfg):
    nc = bacc.Bacc("TRN2", target_bir_lowering=False, debug=False,
                   enable_asserts=False, num_devices=cfg["NCORES"])
    ins = {}
    for nm, (shp, dt) in _input_specs(cfg).items():
        ins[nm] = nc.dram_tensor(nm, shp, dt, kind="ExternalInput").ap()
    outs = dict(
        out_s=nc.dram_tensor("out_s", [128, cfg["NLOC"]], F32,
                             kind="ExternalOutput").ap(),
        out_v=nc.dram_tensor("out_v", [128, 3 * cfg["NLOC"]], F32,
                             kind="ExternalOutput").ap(),
    )
    with tile.TileContext(nc) as tc:
        build_graph(tc, outs, ins, cfg)
    nc.compile()
    return nc


def assemble_outputs(cfg, pp, results):
    N, NLOC = cfg["N"], cfg["NLOC"]
    node_of = pp["node_of"]
    s = np.zeros((N, cfg["F"]), dtype=np.float32)
    v = np.zeros((N, cfg["F"], 3), dtype=np.float32)
    for c, res in enumerate(results):
        sl = node_of[c * NLOC:(c + 1) * NLOC]
        ok = sl >= 0
        s[sl[ok]] = res["out_s"].T[ok]
        vv = res["out_v"].reshape(128, 3, NLOC)
        v[sl[ok]] = vv.transpose(2, 0, 1)[ok]
    return s, v


_CACHED = {}


def kernel(cg_xyz, CG_nbr_list, cg_s, **weights):
    from concourse import bass_utils

    cfg = FULL_CFG
    cg_xyz = np.asarray(cg_xyz, dtype=np.float32)
    cg_s = np.asarray(cg_s, dtype=np.float32)
    nbr = np.asarray(CG_nbr_list).astype(np.int64)
    weights = {k: np.asarray(v, dtype=np.float32) for k, v in weights.items()}

    pp = preprocess(nbr, cfg)
    in_maps = build_inputs(cfg, pp, cg_xyz, cg_s, weights)

    if "nc" not in _CACHED:
        _CACHED["nc"] = build_bass(cfg)
    nc = _CACHED["nc"]
    res = bass_utils.run_bass_kernel_spmd(
        nc, in_maps, core_ids=list(range(cfg["NCORES"])))
    s, v = assemble_outputs(cfg, pp, res.results)
    return s, v


if __name__ == "__main__":
    import reference
    inputs = {k: np.asarray(v) for k, v in reference.setup_inputs().items()}
    out = kernel(**inputs)
    print("kernel ran:", out[0].shape, out[1].shape)
